# revision 54
# baseline (speedup 1.0000x reference)
"""Multi-head attention (B=2, S=2048, D=1024, H=16) on 8 Trainium2 cores, v2.

Sharding: core c -> (batch b = c//4, head-group hg = c%4 of 4 heads, d_h=256).
Megatron-style: column-shard W_{q,k,v}, row-shard W_o; partial outputs summed
on the host.

v2 vs v1: all matmul operands bf16 (PE still 1 row/cycle but halves DMA and
SBUF), exp is one 1024-wide ACT instruction per (kt, head-pair) spanning two
PSUM banks, and the whole kernel is a single software-pipelined stream:
projection matmuls (Q of next q-chunk, O of previous q-chunk, V streamed
during the first q-chunk) are interleaved into the attention loop as PE "pad"
work so the tensor engine never idles (keeps the PE p-state at full clock).
Pipeline per head-pair P over kt slots: scores(kt) -> exp(kt-1) -> PV(kt-8ish)
with the softmax denominator from an appended ones-column in V (VW=65).

v3 vs v2: output in bf16 (halves the 8MB/core out DMA; host sums partials in
f32), weight DMAs on the scalar-engine HWDGE queue in PE need order (SWDGE's
~1us/descriptor trigger cost gated the first projections), prologue x order
K0,Q0,V0,K1-3 matching PE consumption, and a column-half pipelined epilogue:
the last pair drains its final kts eagerly, then exp/PV/normalize/O-proj run
per 256-column half so the halves overlap across ACT/PE/DVE.

PSUM budget (8 banks): scores 2x[128,2,512] (4) + ctx 1x[128,2,512] (2) +
proj 2x[128,512] (2). ctx is freed fast by a DVE copy to SBUF; the
reciprocal/broadcast/normalize chain then runs off-PSUM.
"""

import numpy as np

import bass_rust
import concourse.bass as bass
import concourse.mybir as mybir
import concourse.tile as tile
from concourse.bass_utils import run_bass_kernel_spmd

F32 = mybir.dt.float32
BF16 = mybir.dt.bfloat16
BF16_NP = mybir.dt.np(mybir.dt.bfloat16)

B, S, D = 2, 2048, 1024
H = 16
DK = 64
N_CORES = 8
HEADS_PER_CORE = 4          # d_h = 256
DH = HEADS_PER_CORE * DK    # 256
VW = DK + 1                 # v columns per head incl. ones column
NV = HEADS_PER_CORE * VW    # 260
QC = 512                    # q-chunk (PSUM bank = 512 fp32)
N_QC = S // QC              # 4
N_KT = S // 128             # 16 key tiles
N_KO = D // 128             # 8 contraction tiles for projections
MT = DH // 128              # 2 m-tiles for qT/kT/ctxT
N_PAIR = N_QC * 2           # 8 global (qc, pair) units


def _legalize_waits(nc):
    """walrus here allows 1 sync-wait per instruction (2 for EventSemaphore);
    Tile emits more. Spill extras onto same-engine NoOps placed just before."""
    caps = {"InstEventSemaphore": 2}
    n_nops = 0
    for f in nc.m.functions:
        for bb in f.blocks:
            insts = bb.instructions
            out = []
            changed = False
            for inst in insts:
                si = inst.sync_info
                waits = list(si.on_wait) if si is not None else []
                cap = caps.get(type(inst).__name__, 1)
                if len(waits) > cap:
                    spill, keep = waits[:-cap], waits[-cap:]
                    for w in spill:
                        nop = mybir.InstNoOp(name=f"waitfix-{n_nops}", ins=[], outs=[])
                        n_nops += 1
                        nop.engine = inst.engine
                        nop.sync_info = bass_rust.SyncInfo(on_wait=[w], on_update=[])
                        out.append(nop)
                    si.on_wait = keep
                    changed = True
                out.append(inst)
            if changed:
                insts[:] = out
    return n_nops


def build_nc(reps: int = 1):
    nc = bass.Bass(num_devices=N_CORES)

    t = {}
    t["xqT"] = nc.dram_tensor("xqT", [D, S], BF16, kind="ExternalInput")
    t["xkT"] = nc.dram_tensor("xkT", [D, S], BF16, kind="ExternalInput")
    t["xvT"] = nc.dram_tensor("xvT", [D, S], BF16, kind="ExternalInput")
    t["wqT"] = nc.dram_tensor("wqT", [D, DH], BF16, kind="ExternalInput")
    t["wkT"] = nc.dram_tensor("wkT", [D, DH], BF16, kind="ExternalInput")
    t["wvT"] = nc.dram_tensor("wvT", [D, NV], BF16, kind="ExternalInput")
    t["bq"] = nc.dram_tensor("bq", [DH], F32, kind="ExternalInput")
    t["bk"] = nc.dram_tensor("bk", [DH], F32, kind="ExternalInput")
    t["bv_bc"] = nc.dram_tensor("bv_bc", [128, NV], F32, kind="ExternalInput")
    t["woT"] = nc.dram_tensor("woT", [DH, D], BF16, kind="ExternalInput")
    t["oT"] = nc.dram_tensor("oT", [D, S], BF16, kind="ExternalOutput")

    with tile.TileContext(nc) as tc:
        _body(nc, tc, t, reps)
    _legalize_waits(nc)
    return nc


def _body(nc, tc, t, reps):
    from contextlib import ExitStack

    with ExitStack() as ctx:
        singles = ctx.enter_context(tc.tile_pool(name="singles", bufs=1))

        wq_s = singles.tile([128, N_KO, DH], BF16)
        wk_s = singles.tile([128, N_KO, DH], BF16)
        wv_s = singles.tile([128, N_KO, NV], BF16)
        wo_s = singles.tile([128, MT, D], BF16)
        bq_s = singles.tile([128, MT], F32)
        bk_s = singles.tile([128, MT], F32)
        bv_s = singles.tile([128, NV], F32)
        ones_s = singles.tile([1, 64], F32)
        # all weights on the scalar-engine HWDGE queue (ACT is idle at startup;
        # SWDGE on gpsimd costs ~1us per descriptor and gated the first
        # projections), in PE need order: K proj, Q proj, V proj, O pads last.
        # The sync queue carries the x chunks.
        wk_src = t["wkT"].rearrange("(ko p) m -> p ko m", p=128)
        for ko in range(N_KO):
            nc.scalar.dma_start(wk_s[:, ko], wk_src[:, ko])
        nc.scalar.dma_start(bk_s[:], t["bk"].rearrange("(m p) -> p m", p=128))
        nc.scalar.dma_start(wq_s[:], t["wqT"].rearrange("(ko p) m -> p ko m", p=128))
        nc.scalar.dma_start(bq_s[:], t["bq"].rearrange("(m p) -> p m", p=128))
        nc.scalar.dma_start(wv_s[:], t["wvT"].rearrange("(ko p) m -> p ko m", p=128))
        nc.scalar.dma_start(bv_s[:], t["bv_bc"][:, :])
        nc.scalar.dma_start(wo_s[:], t["woT"].rearrange("(kt p) e -> p kt e", p=128))
        nc.vector.memset(ones_s[:], 1.0)

        qT_s = singles.tile([128, MT, S], BF16)
        kT_s = singles.tile([128, MT, S], BF16)
        v_s = singles.tile([128, N_KT, NV], BF16)
        ctxT_s = singles.tile([128, MT, S], BF16)

        # all pools live across reps so a rep boundary costs only per-tile
        # ring reuse (a per-rep PSUM pool would hold all 8 banks until the
        # previous rep's tail fully drained, serializing the seam)
        pools = {
            "xin": ctx.enter_context(tc.tile_pool(name="xin", bufs=3)),
            "sc_ps": ctx.enter_context(
                tc.tile_pool(name="sc_ps", bufs=2, space="PSUM")),
            "ctx_ps": ctx.enter_context(
                tc.tile_pool(name="ctx_ps", bufs=1, space="PSUM")),
            "pj_ps": ctx.enter_context(
                tc.tile_pool(name="pj_ps", bufs=2, space="PSUM")),
            "pT": ctx.enter_context(tc.tile_pool(name="pT", bufs=10)),
            "csb": ctx.enter_context(tc.tile_pool(name="csb", bufs=2)),
            "nrm": ctx.enter_context(tc.tile_pool(name="nrm", bufs=4)),
            "nd": ctx.enter_context(
                tc.tile_pool(name="nd", bufs=4, space="DRAM")),
            "osb": ctx.enter_context(tc.tile_pool(name="osb", bufs=3)),
        }

        pro_done = False
        for r in range(reps):
            emit_next = r < reps - 1
            _compute(nc, tc, pools, t, wq_s, wk_s, wv_s, bq_s, bk_s, bv_s,
                     wo_s, qT_s, kT_s, v_s, ctxT_s, ones_s,
                     pro_done=pro_done, emit_next_pro=emit_next)
            pro_done = emit_next


def _compute(nc, tc, pools, t, wq_s, wk_s, wv_s, bq_s, bk_s, bv_s, wo_s,
             qT_s, kT_s, v_s, ctxT_s, ones_s, pro_done=False,
             emit_next_pro=False):
    xin = pools["xin"]
    sc_ps = pools["sc_ps"]
    ctx_ps = pools["ctx_ps"]
    pj_ps = pools["pj_ps"]
    pT_pool = pools["pT"]
    csb_pool = pools["csb"]
    nrm_pool = pools["nrm"]
    nd_pool = pools["nd"]
    osb_pool = pools["osb"]

    def dma_x_chunk(xdram, sc, splits=1):
        xt = xin.tile([128, N_KO, QC], BF16, tag="x", name="xt")
        src = xdram.rearrange("(ko p) s -> p ko s", p=128)[
            :, :, sc * QC:(sc + 1) * QC
        ]
        if splits > 1:
            # several DMAs so matmuls over early ko tiles start sooner
            g = N_KO // splits
            for i in range(splits):
                nc.sync.dma_start(xt[:, i * g:(i + 1) * g], src[:, i * g:(i + 1) * g])
        else:
            nc.sync.dma_start(xt[:], src)
        return xt

    # ---------- prologue: K0, Q0, V0, K1-3 (PE order == DMA order) ----------
    if True:
        def pro_qk_chunk(xdram, w_s, b_s, dst, sc, splits=1):
            xt = dma_x_chunk(xdram, sc, splits=splits)
            for m in range(MT):
                # sc-pool tiles: their ring slots free mid-epilogue (last
                # scores/exp), so the next rep's prologue never waits on the
                # previous rep's O tail
                ps = sc_ps.tile([128, 2, QC], F32, tag="sc", name="pro_ps")[:, 0]
                for ko in range(N_KO):
                    nc.tensor.matmul(
                        ps[:],
                        w_s[:, ko, m * 128:(m + 1) * 128],
                        xt[:, ko],
                        start=(ko == 0),
                        stop=(ko == N_KO - 1),
                    )
                nc.vector.tensor_tensor(
                    dst[:, m, sc * QC:(sc + 1) * QC],
                    ps[:],
                    b_s[:, m, None].to_broadcast((128, QC)),
                    mybir.AluOpType.add,
                )

        def pro_v_chunk(sc, splits=1):
            xt = dma_x_chunk(t["xvT"], sc, splits=splits)
            for rt in range(QC // 128):
                ps = sc_ps.tile([128, 2, QC], F32, tag="sc", name="pro_psv")[:, 0]
                for ko in range(N_KO):
                    nc.tensor.matmul(
                        ps[:, :NV],
                        xt[:, ko, rt * 128:(rt + 1) * 128],
                        wv_s[:, ko],
                        start=(ko == 0),
                        stop=(ko == N_KO - 1),
                    )
                nc.vector.tensor_tensor(
                    v_s[:, sc * (QC // 128) + rt, :],
                    ps[:, :NV],
                    bv_s[:],
                    mybir.AluOpType.add,
                )

        if not pro_done:
            # K0/Q0/V0 not pre-projected by the previous rep's epilogue
            pro_qk_chunk(t["xkT"], wk_s, bk_s, kT_s, 0, splits=2)
            pro_qk_chunk(t["xqT"], wq_s, bq_s, qT_s, 0, splits=2)
            pro_v_chunk(0, splits=2)
        for sc in range(1, N_QC):
            pro_qk_chunk(t["xkT"], wk_s, bk_s, kT_s, sc, splits=2)

    # ---------- fused attention + streamed projections ----------
    if True:
        # ----- pad closures: streamed projection work -----
        # pro=True draws PSUM from sc_ps (slots free mid-epilogue), used when
        # the next rep's prologue chunks are woven into this rep's epilogue
        def qk_chunk_closures(xdram, w_s, b_s, dst, sc, pro=False):
            st = {}
            cls = []

            def mk(m, g):
                def f():
                    if m == 0 and g == 0:
                        st["xt"] = dma_x_chunk(xdram, sc)
                    if g == 0:
                        st["ps"] = (
                            sc_ps.tile([128, 2, QC], F32, tag="sc",
                                       name="pjq")[:, 0]
                            if pro else
                            pj_ps.tile([128, QC], F32, tag="pj", name="pjq")
                        )
                    for ko in (2 * g, 2 * g + 1):
                        nc.tensor.matmul(
                            st["ps"][:],
                            w_s[:, ko, m * 128:(m + 1) * 128],
                            st["xt"][:, ko],
                            start=(ko == 0),
                            stop=(ko == N_KO - 1),
                        )
                    if g == 3:
                        nc.vector.tensor_tensor(
                            dst[:, m, sc * QC:(sc + 1) * QC],
                            st["ps"][:],
                            b_s[:, m, None].to_broadcast((128, QC)),
                            mybir.AluOpType.add,
                        )
                return f

            for m in range(MT):
                for g in range(4):
                    cls.append(mk(m, g))
            return cls

        def q_chunk_closures(sc):
            return qk_chunk_closures(t["xqT"], wq_s, bq_s, qT_s, sc)

        def v_chunk_closures(sc, pro=False):
            st = {}
            cls = []

            def mk(rt, g):
                def f():
                    if rt == 0 and g == 0:
                        st["xt"] = dma_x_chunk(t["xvT"], sc)
                    if g == 0:
                        st["ps"] = (
                            sc_ps.tile([128, 2, QC], F32, tag="sc",
                                       name="pjv")[:, 0]
                            if pro else
                            pj_ps.tile([128, QC], F32, tag="pj", name="pjv")
                        )
                    for ko in range(4 * g, 4 * g + 4):
                        nc.tensor.matmul(
                            st["ps"][:, :NV],
                            st["xt"][:, ko, rt * 128:(rt + 1) * 128],
                            wv_s[:, ko],
                            start=(ko == 0),
                            stop=(ko == N_KO - 1),
                        )
                    if g == 1:
                        nc.vector.tensor_tensor(
                            v_s[:, sc * (QC // 128) + rt, :],
                            st["ps"][:, :NV],
                            bv_s[:],
                            mybir.AluOpType.add,
                        )
                return f

            for rt in range(QC // 128):
                for g in range(2):
                    cls.append(mk(rt, g))
            return cls

        def o_chunk_closures(off, w):
            st = {}
            cls = []

            def mk(mt):
                def f():
                    if mt == 0:
                        st["o"] = osb_pool.tile(
                            [128, D // 128, QC], BF16, tag="o", name="o_sb"
                        )
                    ps = pj_ps.tile([128, QC], F32, tag="pj", name="pjo")
                    for kt in range(MT):
                        nc.tensor.matmul(
                            ps[:, :w],
                            wo_s[:, kt, mt * 128:(mt + 1) * 128],
                            ctxT_s[:, kt, off:off + w],
                            start=(kt == 0),
                            stop=(kt == MT - 1),
                        )
                    nc.vector.tensor_copy(st["o"][:, mt, :w], ps[:, :w])
                    if mt == D // 128 - 1:
                        nc.sync.dma_start(
                            t["oT"].rearrange("(mt p) s -> p mt s", p=128)[
                                :, :, off:off + w
                            ],
                            st["o"][:, :, :w],
                        )
                return f

            for mt in range(D // 128):
                cls.append(mk(mt))
            return cls

        def o_chunk_tail(off, w):
            """Latency-optimized O-proj for the final q-chunk: copies
            alternate DVE/ACT and the output DMA goes out per 2 m-tiles so
            nothing big trails the last matmul."""
            o_sb = osb_pool.tile([128, D // 128, QC], BF16, tag="o", name="o_sbt")
            for mt in range(D // 128):
                ps = pj_ps.tile([128, QC], F32, tag="pj", name="pjot")
                for kt in range(MT):
                    nc.tensor.matmul(
                        ps[:, :w],
                        wo_s[:, kt, mt * 128:(mt + 1) * 128],
                        ctxT_s[:, kt, off:off + w],
                        start=(kt == 0),
                        stop=(kt == MT - 1),
                    )
                if mt % 2 == 0:
                    nc.vector.tensor_copy(o_sb[:, mt, :w], ps[:, :w])
                else:
                    nc.scalar.activation(
                        o_sb[:, mt, :w], ps[:, :w],
                        mybir.ActivationFunctionType.Copy,
                    )
                    nc.sync.dma_start(
                        t["oT"].rearrange("(mt p) s -> p mt s", p=128)[
                            :, mt - 1:mt + 1, off:off + w
                        ],
                        o_sb[:, mt - 1:mt + 1, :w],
                    )

        ATT_CHUNKS = [(0, 512), (512, 512), (1024, 512), (1536, 512)]
        N_G = len(ATT_CHUNKS)

        # per-group pad schedule: 32 slots per group; closures in [lo, hi)
        pad_sched = {g: [[] for _ in range(32)] for g in range(N_G)}

        def place(g, cls, lo, hi):
            n = len(cls)
            span = hi - lo
            for i, c in enumerate(cls):
                pad_sched[g][lo + i * span // n].append(c)

        place(0, v_chunk_closures(1), 0, 6)
        place(0, v_chunk_closures(2), 6, 12)
        place(0, v_chunk_closures(3), 12, 18)
        place(0, q_chunk_closures(1), 18, 32)
        place(1, q_chunk_closures(2), 0, 15)
        o0 = o_chunk_closures(0, 512)
        place(1, o0[:6], 8, 32)
        place(2, q_chunk_closures(3), 0, 15)
        o1 = o_chunk_closures(512, 512)
        place(2, o1[:6], 8, 32)
        # group 3's pad supply is thinnest (only O2) and its slots go
        # ACT-paced; donate O1's (early half) and O0's (late half, P7's
        # region) last two m-tiles. Three o_sb staging tiles then coexist
        # until their group-3 flushes -> osb pool needs bufs=3.
        place(3, o1[6:], 2, 8)
        o2 = o_chunk_closures(1024, 512)
        place(3, o2[:6], 8, 32)
        place(3, o0[6:], 16, 24)
        epi_o2 = o2[6:]

        # ----- attention pipeline primitives -----
        # units: (chunk offset, chunk width, pair) — pair selects the head
        # pair (MT tile); two units per chunk group
        UNITS = [(off, w, pair) for off, w in ATT_CHUNKS for pair in range(2)]
        N_U = len(UNITS)

        sc_tiles = {}
        pt_tiles = {}
        ctx_tiles = {}

        def scores(P, kt):
            off, w, pair = UNITS[P]
            stile = sc_ps.tile([128, 2, QC], F32, tag="sc", name="sc")
            for hl in range(2):
                po = 64 * hl
                nc.tensor.matmul(
                    stile[:, hl, :w],
                    kT_s[po:po + 64, pair, kt * 128:(kt + 1) * 128],
                    qT_s[po:po + 64, pair, off:off + w],
                    start=True,
                    stop=True,
                )
            sc_tiles[(P, kt)] = stile

        def expf(P, kt, c0=0, c1=None):
            w = UNITS[P][1]
            c1 = w if c1 is None else c1
            if (P, kt) in sc_tiles:
                stile = sc_tiles.pop((P, kt))
                pt_tiles[(P, kt)] = (
                    pT_pool.tile([128, 2, QC], BF16, tag="p", name="pt"), stile
                )
            pt, stile = pt_tiles[(P, kt)]
            nc.scalar.activation(pt[:, :, c0:c1], stile[:, :, c0:c1],
                                 mybir.ActivationFunctionType.Exp)

        def pv(P, kt, c0=0, c1=None):
            off, w, pair = UNITS[P]
            c1 = w if c1 is None else c1
            if kt == 0:
                ctx_tiles[P] = ctx_ps.tile([128, 2, QC], F32, tag="ctx", name="ctx")
            ct = ctx_tiles[P]
            pt, _ = pt_tiles[(P, kt)]
            for hl in range(2):
                h = 2 * pair + hl
                nc.tensor.matmul(
                    ct[0:VW, hl, c0:c1],
                    v_s[:, kt, h * VW:(h + 1) * VW],
                    pt[:, hl, c0:c1],
                    start=(kt == 0),
                    stop=(kt == N_KT - 1),
                    skip_group_check=(c0 != 0 or c1 != w),
                )
            if c1 == w:
                pt_tiles.pop((P, kt))

        def norm_chain(P):
            off, w, pair = UNITS[P]
            ct = ctx_tiles.pop(P)
            csb = csb_pool.tile([VW, 2, QC], F32, tag="c", name="csb")
            nc.vector.tensor_copy(csb[:, :, :w], ct[0:VW, :, :w])
            r_s = nrm_pool.tile([1, 2, QC], F32, tag="r", name="r_s")
            nc.vector.reciprocal(r_s[:, :, :w], csb[64:65, :, :w])
            r_d = nd_pool.tile([1, 2, QC], F32, tag="rd", name="r_d")
            nc.gpsimd.dma_start(r_d[:, :, :w], r_s[:, :, :w])
            rbc = nrm_pool.tile([64, 2, QC], F32, tag="rbc", name="rbc")
            r_d_sl = r_d[:, :, :w]
            nc.gpsimd.dma_start(
                rbc[:, :, :w],
                bass.AP(
                    tensor=r_d_sl.tensor,
                    offset=r_d_sl.offset,
                    ap=[[0, 64]] + list(r_d_sl.ap[1:]),
                ),
            )
            for hl in range(2):
                nc.vector.tensor_tensor(
                    ctxT_s[64 * hl:64 * hl + 64, pair, off:off + w],
                    csb[0:64, hl, :w],
                    rbc[:, hl, :w],
                    mybir.AluOpType.mult,
                )

        def norm_chain_fast(P, c0, c1):
            """Last-pair norm for columns [c0, c1): reciprocal straight off
            ctx PSUM, then a PE ones-matmul partition-broadcast into a free
            pj bank instead of the DRAM-roundtrip DMA."""
            off, w, pair = UNITS[P]
            cw = c1 - c0
            ct = ctx_tiles[P]
            rbc = pj_ps.tile([128, QC], F32, tag="pj", name="rbcf")
            r_s = nrm_pool.tile([1, 2, QC], F32, tag="r", name="r_sf")
            nc.vector.reciprocal(r_s[:, :, c0:c1], ct[64:65, :, c0:c1])
            csb = csb_pool.tile([64, 2, QC], F32, tag="c", name="csbf")
            nc.vector.tensor_copy(csb[:, :, c0:c1], ct[0:64, :, c0:c1])
            for hl in range(2):
                nc.tensor.matmul(
                    rbc[0:64, hl * cw:(hl + 1) * cw],
                    ones_s[:],
                    r_s[:, hl, c0:c1],
                    start=True,
                    stop=True,
                    skip_group_check=True,
                )
            # DVE may read at most one PSUM operand: csb is SBUF, rbc PSUM
            for hl in range(2):
                nc.vector.tensor_tensor(
                    ctxT_s[64 * hl:64 * hl + 64, pair, off + c0:off + c1],
                    csb[:, hl, c0:c1],
                    rbc[0:64, hl * cw:(hl + 1) * cw],
                    mybir.AluOpType.mult,
                )

        def pv_list(P, tt):
            out = []
            if tt == 8:
                out += [(P, 0), (P, 1)]
            elif tt == 9:
                out += [(P, 2), (P, 3)]
            elif 10 <= tt <= 15:
                out.append((P, tt - 6))
                # last unit: nothing follows, so drain its high kts eagerly
                if P == N_U - 1 and tt >= 12:
                    out.append((P, tt - 2))
            if 0 <= tt <= 5 and P >= 1:
                out.append((P - 1, tt + 10))
            return out

        # ----- driver -----
        for P in range(N_U):
            pair = P % 2
            sched = pad_sched[P // 2]
            for tt in range(16):
                slot = pair * 16 + tt
                scores(P, tt)
                for c in sched[slot]:
                    c()
                if tt >= 1:
                    expf(P, tt - 1)
                elif P >= 1:
                    expf(P - 1, 15)
                if P >= 1 and tt == 6:
                    norm_chain(P - 1)
                for (pp, kk) in pv_list(P, tt):
                    pv(pp, kk)
        # ---- epilogue: column-half pipelined drain of the last unit ----
        # pending: exp(15), pv(14), pv(15), norm, O-proj, out DMA; each is
        # split into column halves so the halves overlap across engines
        PL = N_U - 1
        off_l, w_l, _ = UNITS[PL]
        hw = w_l // 2
        if emit_next_pro:
            # weave the NEXT rep's K0/Q0/V0 projections into this epilogue:
            # they pad the exp/norm/O-tail waits, and their kT/qT/v_s writes
            # only race reads that are already emitted (K0/Q0 cols are last
            # read by P7's early scores; V0's kts by the pv calls above)
            k0c = qk_chunk_closures(t["xkT"], wk_s, bk_s, kT_s, 0, pro=True)
            q0c = qk_chunk_closures(t["xqT"], wq_s, bq_s, qT_s, 0, pro=True)
            v0c = v_chunk_closures(0, pro=True)
        else:
            k0c = q0c = v0c = [lambda: None] * 8
        expf(PL, 15, 0, hw)
        pv(PL, 14, 0, hw)
        k0c[0]()
        k0c[1]()
        if epi_o2:
            epi_o2[0]()
        pv(PL, 15, 0, hw)
        k0c[2]()
        k0c[3]()
        expf(PL, 15, hw, w_l)
        norm_chain_fast(PL, 0, hw)
        k0c[4]()
        k0c[5]()
        pv(PL, 14, hw, w_l)
        for c in epi_o2[1:]:
            c()
        k0c[6]()
        k0c[7]()
        pv(PL, 15, hw, w_l)
        norm_chain_fast(PL, hw, w_l)
        ctx_tiles.pop(PL)
        for c in q0c[:4]:
            c()
        o_chunk_tail(off_l, hw)
        for c in q0c[4:]:
            c()
        o_chunk_tail(off_l + hw, hw)
        for c in v0c:
            c()


def shard_inputs(Q, K, V, Wq, bq, Wk, bk, Wv, bv, Wo, bo):
    """Host-side shard prep. Returns per-core in_maps."""
    scale = 1.0 / np.sqrt(np.float32(DK))
    in_maps = []
    xT = {}
    for b in range(B):
        xT[b] = (
            np.ascontiguousarray(np.asarray(Q[b]).T.astype(BF16_NP)),
            np.ascontiguousarray(np.asarray(K[b]).T.astype(BF16_NP)),
            np.ascontiguousarray(np.asarray(V[b]).T.astype(BF16_NP)),
        )
    for c in range(N_CORES):
        b, hg = c // HEADS_PER_CORE, c % HEADS_PER_CORE
        rows = slice(DH * hg, DH * (hg + 1))
        wqT = np.ascontiguousarray(np.asarray(Wq)[rows].T.astype(BF16_NP))
        wkT = np.ascontiguousarray(
            (np.asarray(Wk)[rows] * scale).T.astype(BF16_NP)
        )
        wvT = np.zeros((D, NV), BF16_NP)
        bv_bc = np.zeros((128, NV), np.float32)
        for i in range(HEADS_PER_CORE):
            wr = slice(DH * hg + DK * i, DH * hg + DK * (i + 1))
            wvT[:, VW * i:VW * i + DK] = np.asarray(Wv)[wr].T.astype(BF16_NP)
            bv_bc[:, VW * i:VW * i + DK] = np.asarray(bv)[wr][None, :]
            bv_bc[:, VW * i + DK] = 1.0
        woT = np.ascontiguousarray(np.asarray(Wo)[:, rows].T.astype(BF16_NP))
        in_maps.append(
            {
                "xqT": xT[b][0],
                "xkT": xT[b][1],
                "xvT": xT[b][2],
                "wqT": wqT,
                "wkT": wkT,
                "wvT": wvT,
                "bq": np.ascontiguousarray(np.asarray(bq)[rows]).astype(np.float32),
                "bk": np.ascontiguousarray(
                    np.asarray(bk)[rows] * scale
                ).astype(np.float32),
                "bv_bc": bv_bc,
                "woT": woT,
            }
        )
    return in_maps


def unshard(results, bo):
    out = np.empty((B, S, D), np.float32)
    for b in range(B):
        acc = results[b * HEADS_PER_CORE]["oT"].astype(np.float32).copy()
        for hg in range(1, HEADS_PER_CORE):
            acc += results[b * HEADS_PER_CORE + hg]["oT"].astype(np.float32)
        out[b] = acc.T + np.asarray(bo)[None, :]
    return out


_NC_CACHE = {}


def kernel(Q, K, V, Wq, bq, Wk, bk, Wv, bv, Wo, bo):
    if "nc" not in _NC_CACHE:
        _NC_CACHE["nc"] = build_nc()
    nc = _NC_CACHE["nc"]
    in_maps = shard_inputs(Q, K, V, Wq, bq, Wk, bk, Wv, bv, Wo, bo)
    res = run_bass_kernel_spmd(nc, in_maps, core_ids=list(range(N_CORES)))
    return unshard(res.results, bo)



# revision 58
# speedup vs baseline: 1.1886x; 1.1886x over previous
"""Multi-head attention (B=2, S=2048, D=1024, H=16) on 8 Trainium2 cores, v2.

Sharding: core c -> (batch b = c//4, head-group hg = c%4 of 4 heads, d_h=256).
Megatron-style: column-shard W_{q,k,v}, row-shard W_o; partial outputs summed
on the host.

v2 vs v1: all matmul operands bf16 (PE still 1 row/cycle but halves DMA and
SBUF), exp is one 1024-wide ACT instruction per (kt, head-pair) spanning two
PSUM banks, and the whole kernel is a single software-pipelined stream:
projection matmuls (Q of next q-chunk, O of previous q-chunk, V streamed
during the first q-chunk) are interleaved into the attention loop as PE "pad"
work so the tensor engine never idles (keeps the PE p-state at full clock).
Pipeline per head-pair P over kt slots: scores(kt) -> exp(kt-1) -> PV(kt-8ish)
with the softmax denominator from an appended ones-column in V (VW=65).

v3 vs v2: output in bf16 (halves the 8MB/core out DMA; host sums partials in
f32), weight DMAs on the scalar-engine HWDGE queue in PE need order (SWDGE's
~1us/descriptor trigger cost gated the first projections), prologue x order
K0,Q0,V0,K1-3 matching PE consumption, and a column-half pipelined epilogue:
the last pair drains its final kts eagerly, then exp/PV/normalize/O-proj run
per 256-column half so the halves overlap across ACT/PE/DVE.

PSUM budget (8 banks): scores 2x[128,2,512] (4) + ctx 1x[128,2,512] (2) +
proj 2x[128,512] (2). ctx is freed fast by a DVE copy to SBUF; the
reciprocal/broadcast/normalize chain then runs off-PSUM.
"""

import numpy as np

import bass_rust
import concourse.bass as bass
import concourse.mybir as mybir
import concourse.tile as tile
from concourse.bass_utils import run_bass_kernel_spmd

F32 = mybir.dt.float32
BF16 = mybir.dt.bfloat16
BF16_NP = mybir.dt.np(mybir.dt.bfloat16)

B, S, D = 2, 2048, 1024
H = 16
DK = 64
N_CORES = 8
HEADS_PER_CORE = 4          # d_h = 256
DH = HEADS_PER_CORE * DK    # 256
VW = DK + 1                 # v columns per head incl. ones column
NV = HEADS_PER_CORE * VW    # 260
QC = 512                    # q-chunk (PSUM bank = 512 fp32)
N_QC = S // QC              # 4
N_KT = S // 128             # 16 key tiles
N_KO = D // 128             # 8 contraction tiles for projections
MT = DH // 128              # 2 m-tiles for qT/kT/ctxT
N_PAIR = N_QC * 2           # 8 global (qc, pair) units


def _legalize_waits(nc):
    """walrus here allows 1 sync-wait per instruction (2 for EventSemaphore);
    Tile emits more. Spill extras onto same-engine NoOps placed just before."""
    caps = {"InstEventSemaphore": 2}
    n_nops = 0
    for f in nc.m.functions:
        for bb in f.blocks:
            insts = bb.instructions
            out = []
            changed = False
            for inst in insts:
                si = inst.sync_info
                waits = list(si.on_wait) if si is not None else []
                cap = caps.get(type(inst).__name__, 1)
                if len(waits) > cap:
                    spill, keep = waits[:-cap], waits[-cap:]
                    for w in spill:
                        nop = mybir.InstNoOp(name=f"waitfix-{n_nops}", ins=[], outs=[])
                        n_nops += 1
                        nop.engine = inst.engine
                        nop.sync_info = bass_rust.SyncInfo(on_wait=[w], on_update=[])
                        out.append(nop)
                    si.on_wait = keep
                    changed = True
                out.append(inst)
            if changed:
                insts[:] = out
    return n_nops


def build_nc(reps: int = 1):
    nc = bass.Bass(num_devices=N_CORES)

    t = {}
    t["xqT"] = nc.dram_tensor("xqT", [D, S], BF16, kind="ExternalInput")
    t["xkT"] = nc.dram_tensor("xkT", [D, S], BF16, kind="ExternalInput")
    t["xvT"] = nc.dram_tensor("xvT", [D, S], BF16, kind="ExternalInput")
    t["wqT"] = nc.dram_tensor("wqT", [D, DH], BF16, kind="ExternalInput")
    t["wkT"] = nc.dram_tensor("wkT", [D, DH], BF16, kind="ExternalInput")
    t["wvT"] = nc.dram_tensor("wvT", [D, NV], BF16, kind="ExternalInput")
    t["bq"] = nc.dram_tensor("bq", [DH], F32, kind="ExternalInput")
    t["bk"] = nc.dram_tensor("bk", [DH], F32, kind="ExternalInput")
    t["bv_bc"] = nc.dram_tensor("bv_bc", [128, NV], F32, kind="ExternalInput")
    t["woT"] = nc.dram_tensor("woT", [DH, D], BF16, kind="ExternalInput")
    t["oT"] = nc.dram_tensor("oT", [D, S], BF16, kind="ExternalOutput")

    with tile.TileContext(nc) as tc:
        _body(nc, tc, t, reps)
    _legalize_waits(nc)
    return nc


def _body(nc, tc, t, reps):
    from contextlib import ExitStack

    with ExitStack() as ctx:
        singles = ctx.enter_context(tc.tile_pool(name="singles", bufs=1))

        wq_s = singles.tile([128, N_KO, DH], BF16)
        wk_s = singles.tile([128, N_KO, DH], BF16)
        wv_s = singles.tile([128, N_KO, NV], BF16)
        wo_s = singles.tile([128, MT, D], BF16)
        bq_s = singles.tile([128, MT], F32)
        bk_s = singles.tile([128, MT], F32)
        bv_s = singles.tile([128, NV], F32)
        ones_s = singles.tile([1, 64], F32)
        # all weights on the scalar-engine HWDGE queue (ACT is idle at startup;
        # SWDGE on gpsimd costs ~1us per descriptor and gated the first
        # projections), in PE need order: K proj, Q proj, V proj, O pads last.
        # The sync queue carries the x chunks.
        wk_src = t["wkT"].rearrange("(ko p) m -> p ko m", p=128)
        for ko in range(N_KO):
            nc.scalar.dma_start(wk_s[:, ko], wk_src[:, ko])
        nc.scalar.dma_start(bk_s[:], t["bk"].rearrange("(m p) -> p m", p=128))
        nc.scalar.dma_start(wq_s[:], t["wqT"].rearrange("(ko p) m -> p ko m", p=128))
        nc.scalar.dma_start(bq_s[:], t["bq"].rearrange("(m p) -> p m", p=128))
        nc.scalar.dma_start(wv_s[:], t["wvT"].rearrange("(ko p) m -> p ko m", p=128))
        nc.scalar.dma_start(bv_s[:], t["bv_bc"][:, :])
        nc.scalar.dma_start(wo_s[:], t["woT"].rearrange("(kt p) e -> p kt e", p=128))
        nc.vector.memset(ones_s[:], 1.0)

        qT_s = singles.tile([128, MT, S], BF16)
        kT_s = singles.tile([128, MT, S], BF16)
        v_s = singles.tile([128, N_KT, NV], BF16)
        ctxT_s = singles.tile([128, MT, S], BF16)

        # all pools live across reps so a rep boundary costs only per-tile
        # ring reuse (a per-rep PSUM pool would hold all 8 banks until the
        # previous rep's tail fully drained, serializing the seam)
        pools = {
            "xin": ctx.enter_context(tc.tile_pool(name="xin", bufs=3)),
            "sc_ps": ctx.enter_context(
                tc.tile_pool(name="sc_ps", bufs=2, space="PSUM")),
            "ctx_ps": ctx.enter_context(
                tc.tile_pool(name="ctx_ps", bufs=1, space="PSUM")),
            "pj_ps": ctx.enter_context(
                tc.tile_pool(name="pj_ps", bufs=2, space="PSUM")),
            "pT": ctx.enter_context(tc.tile_pool(name="pT", bufs=10)),
            "csb": ctx.enter_context(tc.tile_pool(name="csb", bufs=2)),
            "nrm": ctx.enter_context(tc.tile_pool(name="nrm", bufs=4)),
            "nd": ctx.enter_context(
                tc.tile_pool(name="nd", bufs=4, space="DRAM")),
            "osb": ctx.enter_context(tc.tile_pool(name="osb", bufs=3)),
        }

        pro_done = False
        for r in range(reps):
            emit_next = r < reps - 1
            _compute(nc, tc, pools, t, wq_s, wk_s, wv_s, bq_s, bk_s, bv_s,
                     wo_s, qT_s, kT_s, v_s, ctxT_s, ones_s,
                     pro_done=pro_done, emit_next_pro=emit_next)
            pro_done = emit_next


def _compute(nc, tc, pools, t, wq_s, wk_s, wv_s, bq_s, bk_s, bv_s, wo_s,
             qT_s, kT_s, v_s, ctxT_s, ones_s, pro_done=False,
             emit_next_pro=False):
    xin = pools["xin"]
    sc_ps = pools["sc_ps"]
    ctx_ps = pools["ctx_ps"]
    pj_ps = pools["pj_ps"]
    pT_pool = pools["pT"]
    csb_pool = pools["csb"]
    nrm_pool = pools["nrm"]
    nd_pool = pools["nd"]
    osb_pool = pools["osb"]

    def dma_x_chunk(xdram, sc, splits=1):
        xt = xin.tile([128, N_KO, QC], BF16, tag="x", name="xt")
        src = xdram.rearrange("(ko p) s -> p ko s", p=128)[
            :, :, sc * QC:(sc + 1) * QC
        ]
        if splits > 1:
            # several DMAs so matmuls over early ko tiles start sooner
            g = N_KO // splits
            for i in range(splits):
                nc.sync.dma_start(xt[:, i * g:(i + 1) * g], src[:, i * g:(i + 1) * g])
        else:
            nc.sync.dma_start(xt[:], src)
        return xt

    # ---------- prologue: K0, Q0, V0, K1-3 (PE order == DMA order) ----------
    if True:
        def pro_qk_chunk(xdram, w_s, b_s, dst, sc, splits=1):
            xt = dma_x_chunk(xdram, sc, splits=splits)
            for m in range(MT):
                # sc-pool tiles: their ring slots free mid-epilogue (last
                # scores/exp), so the next rep's prologue never waits on the
                # previous rep's O tail
                ps = sc_ps.tile([128, 2, QC], F32, tag="sc", name="pro_ps")[:, 0]
                for ko in range(N_KO):
                    nc.tensor.matmul(
                        ps[:],
                        w_s[:, ko, m * 128:(m + 1) * 128],
                        xt[:, ko],
                        start=(ko == 0),
                        stop=(ko == N_KO - 1),
                    )
                nc.vector.tensor_tensor(
                    dst[:, m, sc * QC:(sc + 1) * QC],
                    ps[:],
                    b_s[:, m, None].to_broadcast((128, QC)),
                    mybir.AluOpType.add,
                )

        def pro_v_chunk(sc, splits=1):
            xt = dma_x_chunk(t["xvT"], sc, splits=splits)
            for rt in range(QC // 128):
                ps = sc_ps.tile([128, 2, QC], F32, tag="sc", name="pro_psv")[:, 0]
                for ko in range(N_KO):
                    nc.tensor.matmul(
                        ps[:, :NV],
                        xt[:, ko, rt * 128:(rt + 1) * 128],
                        wv_s[:, ko],
                        start=(ko == 0),
                        stop=(ko == N_KO - 1),
                    )
                nc.vector.tensor_tensor(
                    v_s[:, sc * (QC // 128) + rt, :],
                    ps[:, :NV],
                    bv_s[:],
                    mybir.AluOpType.add,
                )

        if not pro_done:
            # K0/Q0/V0 not pre-projected by the previous rep's epilogue
            pro_qk_chunk(t["xkT"], wk_s, bk_s, kT_s, 0, splits=2)
            pro_qk_chunk(t["xqT"], wq_s, bq_s, qT_s, 0, splits=2)
            pro_v_chunk(0, splits=2)
        for sc in range(1, N_QC):
            pro_qk_chunk(t["xkT"], wk_s, bk_s, kT_s, sc, splits=2)

    # ---------- fused attention + streamed projections ----------
    if True:
        # ----- pad closures: streamed projection work -----
        # pro=True draws PSUM from sc_ps (slots free mid-epilogue), used when
        # the next rep's prologue chunks are woven into this rep's epilogue
        def qk_chunk_closures(xdram, w_s, b_s, dst, sc, pro=False):
            st = {}
            cls = []

            def mk(m, g):
                def f():
                    if m == 0 and g == 0:
                        st["xt"] = dma_x_chunk(xdram, sc)
                    if g == 0:
                        st["ps"] = (
                            sc_ps.tile([128, 2, QC], F32, tag="sc",
                                       name="pjq")[:, 0]
                            if pro else
                            pj_ps.tile([128, QC], F32, tag="pj", name="pjq")
                        )
                    for ko in (2 * g, 2 * g + 1):
                        nc.tensor.matmul(
                            st["ps"][:],
                            w_s[:, ko, m * 128:(m + 1) * 128],
                            st["xt"][:, ko],
                            start=(ko == 0),
                            stop=(ko == N_KO - 1),
                        )
                    if g == 3:
                        nc.vector.tensor_tensor(
                            dst[:, m, sc * QC:(sc + 1) * QC],
                            st["ps"][:],
                            b_s[:, m, None].to_broadcast((128, QC)),
                            mybir.AluOpType.add,
                        )
                return f

            for m in range(MT):
                for g in range(4):
                    cls.append(mk(m, g))
            return cls

        def q_chunk_closures(sc):
            return qk_chunk_closures(t["xqT"], wq_s, bq_s, qT_s, sc)

        def v_chunk_closures(sc, pro=False):
            st = {}
            cls = []

            def mk(rt, g):
                def f():
                    if rt == 0 and g == 0:
                        st["xt"] = dma_x_chunk(t["xvT"], sc)
                    if g == 0:
                        st["ps"] = (
                            sc_ps.tile([128, 2, QC], F32, tag="sc",
                                       name="pjv")[:, 0]
                            if pro else
                            pj_ps.tile([128, QC], F32, tag="pj", name="pjv")
                        )
                    for ko in range(4 * g, 4 * g + 4):
                        nc.tensor.matmul(
                            st["ps"][:, :NV],
                            st["xt"][:, ko, rt * 128:(rt + 1) * 128],
                            wv_s[:, ko],
                            start=(ko == 0),
                            stop=(ko == N_KO - 1),
                        )
                    if g == 1:
                        nc.vector.tensor_tensor(
                            v_s[:, sc * (QC // 128) + rt, :],
                            st["ps"][:, :NV],
                            bv_s[:],
                            mybir.AluOpType.add,
                        )
                return f

            for rt in range(QC // 128):
                for g in range(2):
                    cls.append(mk(rt, g))
            return cls

        def o_chunk_closures(off, w):
            st = {}
            cls = []

            def mk(mt):
                def f():
                    if mt == 0:
                        st["o"] = osb_pool.tile(
                            [128, D // 128, QC], BF16, tag="o", name="o_sb"
                        )
                    ps = pj_ps.tile([128, QC], F32, tag="pj", name="pjo")
                    for kt in range(MT):
                        nc.tensor.matmul(
                            ps[:, :w],
                            wo_s[:, kt, mt * 128:(mt + 1) * 128],
                            ctxT_s[:, kt, off:off + w],
                            start=(kt == 0),
                            stop=(kt == MT - 1),
                        )
                    nc.vector.tensor_copy(st["o"][:, mt, :w], ps[:, :w])
                    if mt == D // 128 - 1:
                        nc.sync.dma_start(
                            t["oT"].rearrange("(mt p) s -> p mt s", p=128)[
                                :, :, off:off + w
                            ],
                            st["o"][:, :, :w],
                        )
                return f

            for mt in range(D // 128):
                cls.append(mk(mt))
            return cls

        def o_chunk_tail(off, w):
            """Latency-optimized O-proj for the final q-chunk: copies
            alternate DVE/ACT and the output DMA goes out per 2 m-tiles so
            nothing big trails the last matmul."""
            o_sb = osb_pool.tile([128, D // 128, QC], BF16, tag="o", name="o_sbt")
            for mt in range(D // 128):
                ps = pj_ps.tile([128, QC], F32, tag="pj", name="pjot")
                for kt in range(MT):
                    nc.tensor.matmul(
                        ps[:, :w],
                        wo_s[:, kt, mt * 128:(mt + 1) * 128],
                        ctxT_s[:, kt, off:off + w],
                        start=(kt == 0),
                        stop=(kt == MT - 1),
                    )
                if mt % 2 == 0:
                    nc.vector.tensor_copy(o_sb[:, mt, :w], ps[:, :w])
                else:
                    nc.scalar.activation(
                        o_sb[:, mt, :w], ps[:, :w],
                        mybir.ActivationFunctionType.Copy,
                    )
                    nc.sync.dma_start(
                        t["oT"].rearrange("(mt p) s -> p mt s", p=128)[
                            :, mt - 1:mt + 1, off:off + w
                        ],
                        o_sb[:, mt - 1:mt + 1, :w],
                    )

        ATT_CHUNKS = [(0, 512), (512, 512), (1024, 512), (1536, 512)]
        N_G = len(ATT_CHUNKS)

        # per-group pad schedule: 32 slots per group; closures in [lo, hi)
        pad_sched = {g: [[] for _ in range(32)] for g in range(N_G)}

        def place(g, cls, lo, hi):
            n = len(cls)
            span = hi - lo
            for i, c in enumerate(cls):
                pad_sched[g][lo + i * span // n].append(c)

        place(0, v_chunk_closures(1), 0, 6)
        place(0, v_chunk_closures(2), 6, 12)
        place(0, v_chunk_closures(3), 12, 18)
        place(0, q_chunk_closures(1), 18, 32)
        place(1, q_chunk_closures(2), 0, 15)
        o0 = o_chunk_closures(0, 512)
        place(1, o0[:6], 8, 32)
        place(2, q_chunk_closures(3), 0, 15)
        o1 = o_chunk_closures(512, 512)
        place(2, o1[:6], 8, 32)
        # group 3's pad supply is thinnest (only O2) and its slots go
        # ACT-paced; donate O1's (early half) and O0's (late half, P7's
        # region) last two m-tiles. Three o_sb staging tiles then coexist
        # until their group-3 flushes -> osb pool needs bufs=3.
        place(3, o1[6:], 2, 8)
        o2 = o_chunk_closures(1024, 512)
        place(3, o2[:6], 8, 32)
        place(3, o0[6:], 16, 24)
        epi_o2 = o2[6:]

        # ----- attention pipeline primitives -----
        # units: (chunk offset, chunk width, pair) — pair selects the head
        # pair (MT tile); two units per chunk group
        UNITS = [(off, w, pair) for off, w in ATT_CHUNKS for pair in range(2)]
        N_U = len(UNITS)

        sc_tiles = {}
        pt_tiles = {}
        ctx_tiles = {}

        def scores(P, kt):
            off, w, pair = UNITS[P]
            stile = sc_ps.tile([128, 2, QC], F32, tag="sc", name="sc")
            for hl in range(2):
                po = 64 * hl
                nc.tensor.matmul(
                    stile[:, hl, :w],
                    kT_s[po:po + 64, pair, kt * 128:(kt + 1) * 128],
                    qT_s[po:po + 64, pair, off:off + w],
                    start=True,
                    stop=True,
                )
            sc_tiles[(P, kt)] = stile

        def expf(P, kt, c0=0, c1=None):
            w = UNITS[P][1]
            c1 = w if c1 is None else c1
            if (P, kt) in sc_tiles:
                stile = sc_tiles.pop((P, kt))
                pt_tiles[(P, kt)] = (
                    pT_pool.tile([128, 2, QC], BF16, tag="p", name="pt"), stile
                )
            pt, stile = pt_tiles[(P, kt)]
            nc.scalar.activation(pt[:, :, c0:c1], stile[:, :, c0:c1],
                                 mybir.ActivationFunctionType.Exp)

        def pv(P, kt, c0=0, c1=None):
            off, w, pair = UNITS[P]
            c1 = w if c1 is None else c1
            if kt == 0:
                ctx_tiles[P] = ctx_ps.tile([128, 2, QC], F32, tag="ctx", name="ctx")
            ct = ctx_tiles[P]
            pt, _ = pt_tiles[(P, kt)]
            for hl in range(2):
                h = 2 * pair + hl
                nc.tensor.matmul(
                    ct[0:VW, hl, c0:c1],
                    v_s[:, kt, h * VW:(h + 1) * VW],
                    pt[:, hl, c0:c1],
                    start=(kt == 0),
                    stop=(kt == N_KT - 1),
                    skip_group_check=(c0 != 0 or c1 != w),
                )
            if c1 == w:
                pt_tiles.pop((P, kt))

        def norm_chain(P):
            off, w, pair = UNITS[P]
            ct = ctx_tiles.pop(P)
            csb = csb_pool.tile([VW, 2, QC], F32, tag="c", name="csb")
            nc.vector.tensor_copy(csb[:, :, :w], ct[0:VW, :, :w])
            r_s = nrm_pool.tile([1, 2, QC], F32, tag="r", name="r_s")
            nc.vector.reciprocal(r_s[:, :, :w], csb[64:65, :, :w])
            r_d = nd_pool.tile([1, 2, QC], F32, tag="rd", name="r_d")
            nc.gpsimd.dma_start(r_d[:, :, :w], r_s[:, :, :w])
            rbc = nrm_pool.tile([64, 2, QC], F32, tag="rbc", name="rbc")
            r_d_sl = r_d[:, :, :w]
            nc.gpsimd.dma_start(
                rbc[:, :, :w],
                bass.AP(
                    tensor=r_d_sl.tensor,
                    offset=r_d_sl.offset,
                    ap=[[0, 64]] + list(r_d_sl.ap[1:]),
                ),
            )
            for hl in range(2):
                nc.vector.tensor_tensor(
                    ctxT_s[64 * hl:64 * hl + 64, pair, off:off + w],
                    csb[0:64, hl, :w],
                    rbc[:, hl, :w],
                    mybir.AluOpType.mult,
                )

        def norm_chain_fast(P, c0, c1):
            """Last-pair norm for columns [c0, c1): reciprocal straight off
            ctx PSUM, then a PE ones-matmul partition-broadcast into a free
            pj bank instead of the DRAM-roundtrip DMA."""
            off, w, pair = UNITS[P]
            cw = c1 - c0
            ct = ctx_tiles[P]
            rbc = pj_ps.tile([128, QC], F32, tag="pj", name="rbcf")
            r_s = nrm_pool.tile([1, 2, QC], F32, tag="r", name="r_sf")
            nc.vector.reciprocal(r_s[:, :, c0:c1], ct[64:65, :, c0:c1])
            csb = csb_pool.tile([64, 2, QC], F32, tag="c", name="csbf")
            nc.vector.tensor_copy(csb[:, :, c0:c1], ct[0:64, :, c0:c1])
            for hl in range(2):
                nc.tensor.matmul(
                    rbc[0:64, hl * cw:(hl + 1) * cw],
                    ones_s[:],
                    r_s[:, hl, c0:c1],
                    start=True,
                    stop=True,
                    skip_group_check=True,
                )
            # DVE may read at most one PSUM operand: csb is SBUF, rbc PSUM
            for hl in range(2):
                nc.vector.tensor_tensor(
                    ctxT_s[64 * hl:64 * hl + 64, pair, off + c0:off + c1],
                    csb[:, hl, c0:c1],
                    rbc[0:64, hl * cw:(hl + 1) * cw],
                    mybir.AluOpType.mult,
                )

        def pv_list(P, tt):
            out = []
            if tt == 8:
                out += [(P, 0), (P, 1)]
            elif tt == 9:
                out += [(P, 2), (P, 3)]
            elif 10 <= tt <= 15:
                out.append((P, tt - 6))
                # last unit: nothing follows, so drain its high kts eagerly
                if P == N_U - 1 and tt >= 12:
                    out.append((P, tt - 2))
            if 0 <= tt <= 5 and P >= 1:
                out.append((P - 1, tt + 10))
            return out

        # ----- driver -----
        for P in range(N_U):
            pair = P % 2
            sched = pad_sched[P // 2]
            for tt in range(16):
                slot = pair * 16 + tt
                scores(P, tt)
                for c in sched[slot]:
                    c()
                if tt >= 1:
                    expf(P, tt - 1)
                elif P >= 1:
                    expf(P - 1, 15)
                if P >= 1 and tt == 6:
                    norm_chain(P - 1)
                for (pp, kk) in pv_list(P, tt):
                    pv(pp, kk)
        # ---- epilogue: column-half pipelined drain of the last unit ----
        # pending: exp(15), pv(14), pv(15), norm, O-proj, out DMA; each is
        # split into column halves so the halves overlap across engines
        PL = N_U - 1
        off_l, w_l, _ = UNITS[PL]
        hw = w_l // 2
        if emit_next_pro:
            # weave the NEXT rep's K0/Q0/V0 projections into this epilogue:
            # they pad the exp/norm/O-tail waits, and their kT/qT/v_s writes
            # only race reads that are already emitted (K0/Q0 cols are last
            # read by P7's early scores; V0's kts by the pv calls above)
            k0c = qk_chunk_closures(t["xkT"], wk_s, bk_s, kT_s, 0, pro=True)
            q0c = qk_chunk_closures(t["xqT"], wq_s, bq_s, qT_s, 0, pro=True)
            v0c = v_chunk_closures(0, pro=True)
        else:
            k0c = q0c = v0c = [lambda: None] * 8
        expf(PL, 15, 0, hw)
        pv(PL, 14, 0, hw)
        k0c[0]()
        k0c[1]()
        if epi_o2:
            epi_o2[0]()
        pv(PL, 15, 0, hw)
        k0c[2]()
        k0c[3]()
        expf(PL, 15, hw, w_l)
        norm_chain_fast(PL, 0, hw)
        k0c[4]()
        k0c[5]()
        pv(PL, 14, hw, w_l)
        for c in epi_o2[1:]:
            c()
        k0c[6]()
        k0c[7]()
        pv(PL, 15, hw, w_l)
        norm_chain_fast(PL, hw, w_l)
        ctx_tiles.pop(PL)
        for c in q0c[:4]:
            c()
        o_chunk_tail(off_l, hw)
        for c in q0c[4:]:
            c()
        o_chunk_tail(off_l + hw, hw)
        for c in v0c:
            c()


def shard_inputs(Q, K, V, Wq, bq, Wk, bk, Wv, bv, Wo, bo):
    """Host-side shard prep. Returns per-core in_maps."""
    scale = 1.0 / np.sqrt(np.float32(DK))
    in_maps = []
    xT = {}
    for b in range(B):
        xT[b] = (
            np.ascontiguousarray(np.asarray(Q[b]).T.astype(BF16_NP)),
            np.ascontiguousarray(np.asarray(K[b]).T.astype(BF16_NP)),
            np.ascontiguousarray(np.asarray(V[b]).T.astype(BF16_NP)),
        )
    for c in range(N_CORES):
        b, hg = c // HEADS_PER_CORE, c % HEADS_PER_CORE
        rows = slice(DH * hg, DH * (hg + 1))
        wqT = np.ascontiguousarray(np.asarray(Wq)[rows].T.astype(BF16_NP))
        wkT = np.ascontiguousarray(
            (np.asarray(Wk)[rows] * scale).T.astype(BF16_NP)
        )
        wvT = np.zeros((D, NV), BF16_NP)
        bv_bc = np.zeros((128, NV), np.float32)
        for i in range(HEADS_PER_CORE):
            wr = slice(DH * hg + DK * i, DH * hg + DK * (i + 1))
            wvT[:, VW * i:VW * i + DK] = np.asarray(Wv)[wr].T.astype(BF16_NP)
            bv_bc[:, VW * i:VW * i + DK] = np.asarray(bv)[wr][None, :]
            bv_bc[:, VW * i + DK] = 1.0
        woT = np.ascontiguousarray(np.asarray(Wo)[:, rows].T.astype(BF16_NP))
        in_maps.append(
            {
                "xqT": xT[b][0],
                "xkT": xT[b][1],
                "xvT": xT[b][2],
                "wqT": wqT,
                "wkT": wkT,
                "wvT": wvT,
                "bq": np.ascontiguousarray(np.asarray(bq)[rows]).astype(np.float32),
                "bk": np.ascontiguousarray(
                    np.asarray(bk)[rows] * scale
                ).astype(np.float32),
                "bv_bc": bv_bc,
                "woT": woT,
            }
        )
    return in_maps


def unshard(results, bo):
    out = np.empty((B, S, D), np.float32)
    for b in range(B):
        acc = results[b * HEADS_PER_CORE]["oT"].astype(np.float32).copy()
        for hg in range(1, HEADS_PER_CORE):
            acc += results[b * HEADS_PER_CORE + hg]["oT"].astype(np.float32)
        out[b] = acc.T + np.asarray(bo)[None, :]
    return out


_NC_CACHE = {}


def kernel(Q, K, V, Wq, bq, Wk, bk, Wv, bv, Wo, bo):
    if "nc" not in _NC_CACHE:
        _NC_CACHE["nc"] = build_nc()
    nc = _NC_CACHE["nc"]
    in_maps = shard_inputs(Q, K, V, Wq, bq, Wk, bk, Wv, bv, Wo, bo)
    res = run_bass_kernel_spmd(nc, in_maps, core_ids=list(range(N_CORES)))
    return unshard(res.results, bo)

